# revision 10
# baseline (speedup 1.0000x reference)
"""MLA attention distributed over 8 TRN2 NeuronCores.

Sharding: tensor-parallel over heads (4 head-groups) x data-parallel over
batch (2). Each core computes, for its (batch, head-group): the shared KV
compression, K/V up-projections for its 4 heads, a host-fused Q projection,
full attention for its 4 heads, and a partial output projection (its heads'
rows of W_O). Host gather sums the 4 partials per batch.

Key optimizations over the straightforward mapping:
  - All projection GEMMs (compression, K/V/Q up-projections, out-projection)
    run as hi-lo fp8 DoubleRow chains: each operand X is shipped/stored as a
    pair (hi, lo) with hi = e4m3(s*X), lo = e4m3(s*X - hi) -- the same byte
    count as bf16 but ~2.5x more accurate -- and X@W is computed as the three
    cross terms hi*hi + hi*lo + lo*hi, each a DoubleRow matmul contracting
    two 128-row planes per instruction.  That is 6 DR instructions per
    512-deep contraction instead of 4 bf16 instructions, at half the
    per-instruction cost: 25% less PE time than bf16 with better accuracy.
    The dropped lo*lo term is O(ulp^2).
  - The Q path is fused on the host: q = x @ (W_DQ @ [W_QR | W_UQ]) --
    one 640-wide GEMM instead of compression + up-projection.
  - Scores run on the PE in fp8e4m3 DoubleRow perf mode: the head's 128
    compressed dims are plane 0 and the shared rope dims are plane 1 of the
    doubled contraction (the rope block of the score matrix is
    head-independent, so it rides along as a second plane instead of a
    second matmul per head). q/k are stored at 8x scale to stay clear of the
    fp8 subnormal floor; the exp activation scale folds the 1/64 back out.
  - exp is batched two key-blocks per activation ([128,1024] from a 2-bank
    PSUM tile) to amortize the Activation engine's fixed access latency;
    with the all-ones attention mask the bias is a scalar 0. (A masked
    input falls back to per-key-block exp with a per-partition bias.)
  - The attention loop is software-pipelined at query-chunk granularity:
    the Q projection of chunk n+1, the out-projection (phase 5) of chunk
    n-1, and rope (on the idle Pool engine) all interleave into chunk n's
    score/AV emission, so the PE never waits on the Activation engine's
    exp round-trip. One accumulation chain per PSUM bank throughout (a
    start=True poisons the whole 2KB zero-region granule).
Attention probabilities and V run in bf16; softmax needs no max-subtraction
(scores are bounded ~|2|) and the denominator comes from a ones-column
appended to V.
"""

from collections import deque
from contextlib import ExitStack

import ml_dtypes
import numpy as np

import concourse.bacc as bacc
import concourse.mybir as mybir
import concourse.tile as tile
from concourse.bass_utils import run_bass_kernel_spmd
from concourse.masks import make_identity

B, L, D, H, DC, DH = 2, 2048, 2048, 16, 512, 128
HG = 4                 # head groups (tensor-parallel degree per batch)
HL = H // HG           # heads per core
HDL = HL * DH          # 512 head-dims per core
P = 128
N1 = 512               # matmul free-dim chunk
F32 = mybir.dt.float32
BF16 = mybir.dt.bfloat16
FP8 = mybir.dt.float8e4
F8NP = ml_dtypes.float8_e4m3
DR = mybir.MatmulPerfMode.DoubleRow
SCALE = 1.0 / float(np.sqrt(2 * DH))
FP8_PRE = 8.0          # q/k fp8 operand scale in kall/qall
XPRE = 16.0            # hi-lo pre-scale for activations (x)
WPRE = 400.0           # hi-lo pre-scale for weights
CTXPRE = 128.0         # hi-lo pre-scale for attention context
M1 = HDL + DH          # 640 fused output rows ([W_KR|W_DKV] / fused-Q)
MT1 = M1 // P          # 5
KT1 = D // P           # 16
NCH = L // N1          # 4 query/seq chunks
KT3 = DC // P          # 4
KB = L // P            # 16 key blocks
KBP = KB // 2          # 8 key-block pairs
DV = DH + 1            # value cols + denominator ones-column
NEG = -30000.0         # additive mask bias for masked-out keys
MULT = mybir.AluOpType.mult
SUB = mybir.AluOpType.subtract
# PSUM scale of the hi-lo projection chains and the derived copy-out scales
S1 = XPRE * WPRE            # ph1a / ph1b accumulate at 6400x
SCK = XPRE                  # on-chip c_kv pair is stored at 16x
S3 = SCK * WPRE             # ph3k / 3v accumulate at 6400x
S5 = CTXPRE * WPRE          # ph5 accumulates at 51200x


def build_nc(mask_ones=True):
    nc = bacc.Bacc(None, target_bir_lowering=False)

    # hi/lo pairs are interleaved innermost: [..., 2] with hi at index 0.
    xhl = nc.dram_tensor("xhl", [D, L * 2], FP8, kind="ExternalInput")
    w1a = nc.dram_tensor("w1a", [D, M1 * 2], FP8, kind="ExternalInput")
    w1b = nc.dram_tensor("w1b", [D, M1 * 2], FP8, kind="ExternalInput")
    wuk = nc.dram_tensor("wuk", [DC, HDL * 2], FP8, kind="ExternalInput")
    wuv = nc.dram_tensor("wuv", [DC, HDL * 2], FP8, kind="ExternalInput")
    wo = nc.dram_tensor("wo", [HDL, D * 2], FP8, kind="ExternalInput")
    cs_d = nc.dram_tensor("csT", [P, L], BF16, kind="ExternalInput")
    mask_d = nc.dram_tensor("maskb", [P, KB], F32, kind="ExternalInput")
    out_d = nc.dram_tensor("out", [L, D], BF16, kind="ExternalOutput")

    with tile.TileContext(nc) as tc, ExitStack() as es:
        # ---------- constants ----------
        p_const = es.enter_context(tc.tile_pool(name="const", bufs=1))
        bias_t = p_const.tile([P, KB], F32, name="bias_t")
        ident = p_const.tile([P, P], BF16, name="ident")
        make_identity(nc, ident[:])
        warm = p_const.tile([P, 1], F32, name="warm")
        nc.scalar.activation(warm[:], bias_t[:, 0:1],
                             mybir.ActivationFunctionType.Exp)


        # ---------- right-side residents (live until end) ----------
        es_tab = ExitStack()
        p_tab = es_tab.enter_context(tc.tile_pool(name="tabp", bufs=1,
                                                  side="right"))
        cos_t = p_tab.tile([DH // 2, L], BF16, name="cos_t")
        sin_t = p_tab.tile([DH // 2, L], BF16, name="sin_t")

        es_xr = ExitStack()
        p_xr = es_xr.enter_context(tc.tile_pool(name="xrp", bufs=1,
                                                side="right"))
        xr_t = p_xr.tile([P, L], BF16, name="xrT")
        xrk_t = xrq_t = xr_t   # xrk is dead before xrq is first written

        es_w1b = ExitStack()
        p_w1b = es_w1b.enter_context(tc.tile_pool(name="w1bp", bufs=1,
                                                  side="right"))
        es_rope = ExitStack()
        p_rope = es_rope.enter_context(tc.tile_pool(name="ropep", bufs=1,
                                                    side="right"))

        # ---------- long-lived left-side pools (bottom of stack) ----------
        # attention operands: planes 0..3 = per-head compressed dims,
        # plane 4 = shared rope dims (packed fp8 for DoubleRow)
        es_att = ExitStack()
        p_att = es_att.enter_context(tc.tile_pool(name="attp", bufs=1))
        kall = p_att.tile([P, HL + 1, L], FP8, name="kall")
        qall = p_att.tile([P, HL + 1, L], FP8, name="qall")
        vaug_t = [p_att.tile([P, HL * DV], BF16, tag=f"v{i}", name=f"v{i}")
                  for i in range(KB)]

        # ---------- transient pools for phases 1a/3 ----------
        # wukv/ckv feed the 3v filler chains inside the attention loop, so
        # they sit below x/w1a and stay open until the end
        es_ps13 = ExitStack()
        p_ps13 = es_ps13.enter_context(tc.tile_pool(name="ps13", bufs=3,
                                                    space="PSUM"))
        es_wukv = ExitStack()
        p_wuk = es_wukv.enter_context(tc.tile_pool(name="wukp", bufs=1))
        p_wuv = es_wukv.enter_context(tc.tile_pool(name="wuvp", bufs=1))
        es_ckv = ExitStack()
        p_ckv = es_ckv.enter_context(tc.tile_pool(name="ckvp", bufs=1))
        ckv2 = p_ckv.tile([P, KT3, L, 2], FP8, name="ckv2")
        es_x = ExitStack()
        p_x = es_x.enter_context(tc.tile_pool(name="xp", bufs=1))
        es_w1a = ExitStack()
        p_w1a = es_w1a.enter_context(tc.tile_pool(name="w1ap", bufs=1))

        # 3-term hi-lo DoubleRow chain: stat/mov are [P, KT, cols, 2] tiles
        # (hi at [..., 0]); each kt-pair contributes hi*hi, hi*lo, lo*hi.
        def hl_chain(ps, stat, scols, mov, mcols, nkt):
            first = True
            for kt in range(0, nkt, 2):
                for (a, b) in ((0, 0), (0, 1), (1, 0)):
                    last = kt == nkt - 2 and (a, b) == (1, 0)
                    nc.tensor.matmul(ps, stat[:, kt:kt + 2, scols, a],
                                     mov[:, kt:kt + 2, mcols, b],
                                     start=first, stop=last, perf_mode=DR)
                    first = False

        # hi-lo split of a PSUM chain result into an interleaved fp8 pair:
        # hi = e4m3(ps*sc), lo = e4m3(ps*sc - hi)
        def hl_split(eng, dst_hi, dst_lo, ps, sc):
            eng.tensor_scalar_mul(dst_hi, ps, sc)
            eng.scalar_tensor_tensor(dst_lo, ps, sc, dst_hi, MULT, SUB)

        # rope in transposed layout: even rows 0:64 / odd rows 64:128 of the
        # pre-roped projection (host permuted the weight columns). Runs on
        # the Pool (gpsimd) engine, per sequence chunk, writing fp8 planes.
        def rope_chunk(src_t, dst_pl, pfx, ch):
            cs = slice(ch * N1, (ch + 1) * N1)
            eng = nc.gpsimd
            xo = p_rope.tile([64, N1], BF16, tag="rxo", name=f"{pfx}xo{ch}")
            eng.dma_start(xo[:], src_t[64:P, cs])
            t1 = p_rope.tile([64, N1], BF16, tag="rt1", name=f"{pfx}t1{ch}")
            t2 = p_rope.tile([64, N1], BF16, tag="rt2", name=f"{pfx}t2{ch}")
            h2 = p_rope.tile([64, N1], FP8, tag="rh2", name=f"{pfx}h2{ch}")
            xe = src_t[0:64, cs]
            cc, ss = cos_t[:, cs], sin_t[:, cs]
            eng.tensor_tensor(t1[:], xe, cc, MULT)
            eng.tensor_tensor(t2[:], xo[:], ss, MULT)
            eng.tensor_tensor(dst_pl[0:64, cs], t1[:], t2[:], SUB)
            t3 = p_rope.tile([64, N1], BF16, tag="rt1", name=f"{pfx}t3{ch}")
            t4 = p_rope.tile([64, N1], BF16, tag="rt2", name=f"{pfx}t4{ch}")
            eng.tensor_tensor(t3[:], xe, ss, MULT)
            eng.tensor_tensor(t4[:], xo[:], cc, MULT)
            eng.tensor_tensor(h2[:], t3[:], t4[:], mybir.AluOpType.add)
            eng.dma_start(dst_pl[64:P, cs], h2[:])

        # ---------- DMA order on the sync queue: strict priority ----------
        # One big strided transfer per logical tensor: HWDGE descriptor
        # generation costs ~630ns per dma_start, so few large calls beat
        # many tile-sized ones. Order: w1a rope-cols, x chunk0, w1a rest,
        # x chunks 1-3, wuk, wuv (w1b/wo queued after the 1a loop).
        xT_r = xhl.rearrange("(k p) (l t) -> p k l t", p=P, t=2)
        w1a_r = w1a.rearrange("(k p) (m t) -> p k m t", p=P, t=2)
        xb2 = p_x.tile([P, KT1, L, 2], FP8, name="xb2")
        w1a2 = p_w1a.tile([P, KT1, M1, 2], FP8, name="w1a2")
        for k0, k1 in ((0, 2), (2, 4), (4, 8), (8, 12)):
            nc.sync.dma_start(w1a2[:, k0:k1, 0:P, :], w1a_r[:, k0:k1, 0:P, :])
            nc.sync.dma_start(xb2[:, k0:k1, 0:N1, :], xT_r[:, k0:k1, 0:N1, :])
        nc.sync.dma_start(w1a2[:, 12:KT1, 0:P, :], w1a_r[:, 12:KT1, 0:P, :])
        nc.sync.dma_start(w1a2[:, 0:8, P:2 * P, :], w1a_r[:, 0:8, P:2 * P, :])
        nc.sync.dma_start(xb2[:, 12:KT1, 0:N1, :], xT_r[:, 12:KT1, 0:N1, :])
        nc.sync.dma_start(w1a2[:, 8:KT1, P:2 * P, :],
                          w1a_r[:, 8:KT1, P:2 * P, :])
        nc.sync.dma_start(w1a2[:, :, 2 * P:3 * P, :],
                          w1a_r[:, :, 2 * P:3 * P, :])
        nc.sync.dma_start(cos_t[:], cs_d[0:DH // 2, :])
        nc.sync.dma_start(w1a2[:, :, 3 * P:4 * P, :], w1a_r[:, :, 3 * P:4 * P, :])
        nc.sync.dma_start(bias_t[:], mask_d[:])
        nc.sync.dma_start(xb2[:, :, N1:2 * N1, :], xT_r[:, :, N1:2 * N1, :])
        nc.sync.dma_start(w1a2[:, :, 4 * P:M1, :], w1a_r[:, :, 4 * P:M1, :])
        nc.sync.dma_start(sin_t[:], cs_d[DH // 2:P, :])
        for nci in range(2, NCH):
            nc.sync.dma_start(xb2[:, :, nci * N1:(nci + 1) * N1, :],
                              xT_r[:, :, nci * N1:(nci + 1) * N1, :])
        wuk2 = p_wuk.tile([P, KT3, HDL, 2], FP8, name="wuk2")
        nc.sync.dma_start(wuk2[:],
                          wuk.rearrange("(k p) (m t) -> p k m t", p=P, t=2))
        wuv2 = p_wuv.tile([P, KT3, HDL, 2], FP8, name="wuv2")
        nc.sync.dma_start(wuv2[:],
                          wuv.rearrange("(k p) (m t) -> p k m t", p=P, t=2))

        # ---------- phase 1a: [xrkT | c_kvT] = [Wkr | Wdkv].T @ x.T ----
        # mt order puts the rope row-block first so each chunk's rope can
        # run on Pool while the PE continues with the c_kv rows.
        MTO = [MT1 - 1] + list(range(MT1 - 1))
        for nci in range(NCH):
            cs = slice(nci * N1, (nci + 1) * N1)
            for sl, mt in enumerate(MTO):
                ps = p_ps13.tile([P, N1], F32, tag="g", name=f"ps1a_{nci}_{mt}")
                hl_chain(ps[:], w1a2, slice(sl * P, (sl + 1) * P),
                         xb2, cs, KT1)
                if mt == MT1 - 1:
                    nc.vector.tensor_scalar_mul(xrk_t[:, cs], ps[:],
                                                FP8_PRE / S1)
                    rope_chunk(xrk_t, kall[:, HL, :], "k", nci)
                else:
                    hl_split(nc.vector, ckv2[:, mt, cs, 0], ckv2[:, mt, cs, 1],
                             ps[:], SCK / S1)
        # w1b prefetch queued behind the x stream
        w1b2 = p_w1b.tile([P, KT1, M1, 2], FP8, name="w1b2")
        nc.sync.dma_start(w1b2[:],
                          w1b.rearrange("(k p) (m t) -> p k m t", p=P, t=2))
        es_w1a.close()

        # ---------- phase 3k: k_cT = Wuk_hg.T @ c_kvT -> fp8 plane ----
        for nci in range(NCH):
            cs = slice(nci * N1, (nci + 1) * N1)
            for h in range(HL):
                ps = p_ps13.tile([P, N1], F32, tag="g", name=f"ps3k_{nci}_{h}")
                hl_chain(ps[:], wuk2, slice(h * P, (h + 1) * P),
                         ckv2, cs, KT3)
                nc.vector.tensor_scalar_mul(kall[:, h, cs], ps[:],
                                            FP8_PRE / S3)

        # ---------- phase 1b: [xrqT | q_cT] = fusedWq.T @ x.T ----------
        qdest = [qall[:, h, :] for h in range(HL)] + [xrq_t[:]]
        for nci in range(NCH):
            cs = slice(nci * N1, (nci + 1) * N1)
            for sl, mt in enumerate(MTO):
                ps = p_ps13.tile([P, N1], F32, tag="g", name=f"ps1b_{nci}_{mt}")
                hl_chain(ps[:], w1b2, slice(sl * P, (sl + 1) * P),
                         xb2, cs, KT1)
                nc.vector.tensor_scalar_mul(qdest[mt][:, cs], ps[:],
                                            FP8_PRE / S1)
                if mt == MT1 - 1:
                    rope_chunk(xrq_t, qall[:, HL, :], "q", nci)
        es_ps13.close()
        es_x.close()

        # ---------- pools for the fused 1b + attention + out-proj --------
        es_wo = ExitStack()
        p_wo = es_wo.enter_context(tc.tile_pool(name="wop", bufs=1,
                                                side="right"))
        wo2 = p_wo.tile([P, HL, D, 2], FP8, name="wo2")
        nc.sync.dma_start(wo2[:],
                          wo.rearrange("(k p) (d t) -> p k d t", p=P, t=2))
        es_ctx = ExitStack()
        p_ctx = es_ctx.enter_context(tc.tile_pool(name="ctxp", bufs=1))
        ctx2 = p_ctx.tile([P, HL, L, 2], FP8, name="ctx2")
        es_p4 = ExitStack()
        p_sc = es_p4.enter_context(tc.tile_pool(name="scp", bufs=2,
                                                space="PSUM"))
        p_av = es_p4.enter_context(tc.tile_pool(name="avp", bufs=2,
                                                space="PSUM"))
        # shared ring for ph5 chains and (bitcast) transpose targets; they
        # never interleave inside one bank
        p_ch = es_p4.enter_context(tc.tile_pool(name="chp", bufs=2,
                                                space="PSUM"))
        p_e = es_p4.enter_context(tc.tile_pool(name="expp", bufs=16))
        p_sm = es_p4.enter_context(tc.tile_pool(name="smallp", bufs=8))
        p_st = es_p4.enter_context(tc.tile_pool(name="stagep", bufs=4))

        escale = float(SCALE / (FP8_PRE * FP8_PRE))

        # -- phase-5 blocks of query chunk n-1, interleaved as fillers --
        # stage a full [P, D] row-block in bf16, one out-DMA per mt (4 calls
        # per query chunk instead of 16 — HWDGE desc-gen is 625ns per call)
        ph5_pending = deque()
        ph5_stage = {}

        def emit_ph5_block(mt, nci, ps):
            hl_chain(ps, ctx2, slice(mt * P, (mt + 1) * P),
                     wo2, slice(nci * N1, (nci + 1) * N1), HL)
            if nci == 0:
                ph5_stage[mt] = p_st.tile([P, D], BF16, tag="stage",
                                          name=f"st_{mt}")
            stg = ph5_stage[mt]
            nc.vector.tensor_scalar_mul(
                stg[:, nci * N1:(nci + 1) * N1], ps, 1.0 / S5)
            if nci == NCH - 1:
                nc.sync.dma_start(out_d[mt * P:(mt + 1) * P, :],
                                  ph5_stage.pop(mt)[:])

        # 3v chains (v = c_kv @ Wuv, bf16 + ones col) run as qch0-hs0
        # fillers: AV first touches vaug at hs1, scores don't need it
        def gen_3v(mt):
            ps = p_ch.tile([P, N1], F32, tag="ch", name=f"psv_{mt}")
            hl_chain(ps[:], ckv2, slice(mt * P, (mt + 1) * P),
                     wuv2, slice(0, HDL), KT3)
            va = vaug_t[mt].rearrange("p (h c) -> p h c", c=DV)
            nc.vector.tensor_scalar_mul(
                va[:, :, 0:DH], ps.rearrange("p (h c) -> p h c", c=DH),
                1.0 / S3)
            nc.vector.memset(va[:, :, DH:DV], 1.0)

        v3_pending = deque(range(KB))

        def filler():
            if v3_pending:
                gen_3v(v3_pending.popleft())
            elif ph5_pending:
                mt, nci = ph5_pending.popleft()
                ps = p_ch.tile([P, N1], F32, tag="ch",
                               name=f"ps5_{mt}_{nci}")
                emit_ph5_block(mt, nci, ps[:])


        # ---------- fused attention + out-projection loop ----------
        et_t = {}              # (h, kbp) -> exp tile, two key-blocks wide
        av_t = {}              # (h, qc) -> psum chain tile [P, N1] (1 bank)

        def av_chain_seg(hp, qc, kbp, half, first, last):
            kb = 2 * kbp + half
            nc.tensor.matmul(
                av_t[(hp, qc)][:, 0:DV],
                et_t[(hp, kbp)][:, half, qc * P:(qc + 1) * P],
                vaug_t[kb][:, hp * DV:(hp + 1) * DV],
                start=first, stop=last)

        def norm(qch, hp, qc):
            pc = av_t.pop((hp, qc))
            rc = p_sm.tile([P, 1], F32, tag="recip", name=f"rc_{qch}_{hp}_{qc}")
            nc.vector.reciprocal(rc[:], pc[:, DH:DV])
            cn = p_sm.tile([P, DH], BF16, tag="cn", name=f"cn_{qch}_{hp}_{qc}")
            nc.vector.tensor_scalar_mul(cn[:], pc[:, 0:DH], rc[:])
            et_t[("cn", hp, qc)] = cn

        for qch in range(NCH):
            qs = slice(qch * N1, (qch + 1) * N1)
            for hs in range(HL + 1):
                # transposes for head hs-2 (deferred so the DVE norm of that
                # head has certainly drained); bitcast [P,N1]f32 chain slots
                # to bf16 for the [P,P] transpose target
                if hs >= 2 or (qch > 0 and hs == 0):
                    hp2 = hs - 2 if hs >= 2 else HL - 1
                    qq = qch if hs >= 2 else qch - 1
                    for qc in range(4):
                        q0 = qq * 4 + qc
                        cn = et_t.pop(("cn", hp2, qc))
                        pt = p_ch.tile([P, N1], F32, tag="ch",
                                       name=f"tp_{qq}_{hp2}_{q0}")
                        ptb = pt[:].bitcast(BF16)[:, 0:P]
                        nc.tensor.transpose(ptb, cn[:], ident[:])
                        hl_split(nc.vector,
                                 ctx2[:, hp2, q0 * P:(q0 + 1) * P, 0],
                                 ctx2[:, hp2, q0 * P:(q0 + 1) * P, 1],
                                 ptb, CTXPRE)
                hp = hs - 1
                if hs > 0:
                    # one PSUM bank per accumulation chain: a start=True in a
                    # bank poisons the whole 2KB zero-region granule, so two
                    # interleaved chains must never share a bank
                    for qc in range(2):
                        av_t[(hp, qc)] = p_av.tile(
                            [P, N1], F32, tag="av", name=f"av_{qch}_{hp}_{qc}")
                # sweep 1: scores/exp of head hs + AV chains qc0/qc1 of
                # hs-1. The AV consumption order is rotated by +2 key-block
                # pairs so the chain starts on already-drained exp tiles
                # (PSUM accumulation is commutative); SC is emitted after
                # the AV/filler work so a psum-slot wait never idles the PE.
                for i in range(KBP):
                    kbp = i
                    if hs > 0:
                        if i % 2 == 1:
                            filler()
                        for half in range(2):
                            av_chain_seg(hp, 0, kbp, half, i == 0 and half == 0,
                                         i == KBP - 1 and half == 1)
                            av_chain_seg(hp, 1, kbp, half, i == 0 and half == 0,
                                         i == KBP - 1 and half == 1)
                    elif qch == 0:
                        filler()
                        filler()
                    else:
                        filler()
                        if i % 2 == 1:
                            filler()
                    if hs < HL:
                        sc = p_sc.tile([P, 2, N1], F32, tag="sc",
                                       name=f"sc_{qch}_{hs}_{i}")
                        et = p_e.tile([P, 2, N1], BF16, tag="expT",
                                      name=f"et_{qch}_{hs}_{i}")
                        for half in range(2):
                            kb = 2 * i + half
                            nc.tensor.matmul(
                                sc[:, half, :],
                                kall[:, hs::(HL - hs), kb * P:(kb + 1) * P],
                                qall[:, hs::(HL - hs), qs],
                                start=True, stop=True, perf_mode=DR)
                        if mask_ones:
                            nc.scalar.activation(
                                et[:], sc[:],
                                mybir.ActivationFunctionType.Exp,
                                scale=escale)
                        else:
                            for half in range(2):
                                kb = 2 * i + half
                                nc.scalar.activation(
                                    et[:, half, :], sc[:, half, :],
                                    mybir.ActivationFunctionType.Exp,
                                    bias=bias_t[:, kb:kb + 1], scale=escale)
                        et_t[(hs, i)] = et
                # sweep 2: AV chains qc2/qc3 of head hs-1 + norms
                if hs > 0:
                    norm(qch, hp, 0)
                    norm(qch, hp, 1)
                    for qc in range(2, 4):
                        av_t[(hp, qc)] = p_av.tile(
                            [P, N1], F32, tag="av", name=f"av_{qch}_{hp}_{qc}")
                    for i in range(KBP):
                        for half in range(2):
                            av_chain_seg(hp, 2, i, half, i == 0 and half == 0,
                                         i == KBP - 1 and half == 1)
                            av_chain_seg(hp, 3, i, half, i == 0 and half == 0,
                                         i == KBP - 1 and half == 1)
                        if i % 2 == 1:
                            filler()
                    for kbp in range(KBP):
                        et_t.pop((hp, kbp))
                    norm(qch, hp, 2)
                    norm(qch, hp, 3)
            ph5_pending.extend((qch * 4 + mt, nci)
                               for mt in range(4) for nci in range(NCH))


        # flush: transposes of the last head + phase 5 for the last chunk;
        # alternate two psum rings (sc is free now) so chains double-buffer
        for qc in range(4):
            q0 = (NCH - 1) * 4 + qc
            cn = et_t.pop(("cn", HL - 1, qc))
            pt = p_ch.tile([P, N1], F32, tag="ch", name=f"tpf_{q0}")
            ptb = pt[:].bitcast(BF16)[:, 0:P]
            nc.tensor.transpose(ptb, cn[:], ident[:])
            hl_split(nc.vector,
                     ctx2[:, HL - 1, q0 * P:(q0 + 1) * P, 0],
                     ctx2[:, HL - 1, q0 * P:(q0 + 1) * P, 1],
                     ptb, CTXPRE)
        fi = 0
        while ph5_pending:
            mt, nci = ph5_pending.popleft()
            if fi % 2 == 0:
                ps = p_sc.tile([P, 2, N1], F32, tag="sc",
                               name=f"ps5f_{fi}")[:, 0, :]
            else:
                ps = p_ch.tile([P, N1], F32, tag="ch", name=f"ps5f_{fi}")[:]
            fi += 1
            emit_ph5_block(mt, nci, ps)

        es_p4.close()
        es_ctx.close()
        es_ckv.close()
        es_wukv.close()
        es_att.close()
        es_wo.close()
        es_rope.close()
        es_w1b.close()
        es_xr.close()
        es_tab.close()

    nc.compile()
    return nc


_CACHE = {}


def _get_nc(mask_ones=True):
    key = ("nc", mask_ones)
    if key not in _CACHE:
        _CACHE[key] = build_nc(mask_ones)
    return _CACHE[key]


def _split8(a, pre):
    """hi-lo e4m3 pair of pre*a, interleaved innermost: [d0, d1*2]."""
    s = np.asarray(a, np.float32) * np.float32(pre)
    hi = s.astype(F8NP)
    lo = (s - hi.astype(np.float32)).astype(F8NP)
    return np.ascontiguousarray(np.stack([hi, lo], axis=-1).reshape(
        a.shape[0], a.shape[1] * 2))


def _host_prep(x, attention_mask, W_DKV, W_DQ, W_UK, W_UV, W_UQ, W_KR, W_QR,
               W_O):
    f = np.float32
    bf = ml_dtypes.bfloat16
    x = np.asarray(x, f)
    attention_mask = np.asarray(attention_mask)
    W_DKV, W_DQ = np.asarray(W_DKV, f), np.asarray(W_DQ, f)
    W_UK, W_UV, W_UQ = np.asarray(W_UK, f), np.asarray(W_UV, f), np.asarray(W_UQ, f)
    W_KR, W_QR, W_O = np.asarray(W_KR, f), np.asarray(W_QR, f), np.asarray(W_O, f)

    perm = np.concatenate([np.arange(0, DH, 2), np.arange(1, DH, 2)])
    # column layout [rope | dkv]: the rope block is computed first on-chip
    w1a = _split8(np.concatenate([W_KR[:, perm], W_DKV], axis=1), WPRE)
    xhls = [_split8(x[b].T, XPRE) for b in range(B)]

    inv = 1.0 / (10000.0 ** (np.arange(0, DH, 2, dtype=f) / DH))
    freqs = np.arange(L, dtype=f)[:, None] * inv[None, :]
    rope = np.concatenate([np.sin(freqs), np.cos(freqs)], axis=-1).astype(f)
    csT = np.ascontiguousarray(np.concatenate(
        [rope[:, 1::2].T, rope[:, 0::2].T], axis=0).astype(bf))

    maskbs = []
    for b in range(B):
        bias = np.where(attention_mask[b] == 0, f(NEG), f(0.0)).astype(f)
        maskbs.append(np.ascontiguousarray(bias.reshape(KB, P).T))

    w1bs, wuks, wuvs, wos = [], [], [], []
    for hg in range(HG):
        cols = slice(hg * HDL, (hg + 1) * HDL)
        wq = W_DQ @ np.concatenate([W_QR[:, perm], W_UQ[:, cols]], axis=1)
        w1bs.append(_split8(wq, WPRE))
        wuks.append(_split8(W_UK[:, cols], WPRE))
        wuvs.append(_split8(W_UV[:, cols], WPRE))
        wos.append(_split8(W_O[hg * HDL:(hg + 1) * HDL, :], WPRE))

    in_maps = []
    for c in range(8):
        b, hg = c // HG, c % HG
        in_maps.append({
            "xhl": xhls[b],
            "w1a": w1a,
            "w1b": w1bs[hg],
            "wuk": wuks[hg],
            "wuv": wuvs[hg],
            "wo": wos[hg],
            "csT": csT,
            "maskb": maskbs[b],
        })
    return in_maps


def kernel(x, attention_mask, W_DKV, W_DQ, W_UK, W_UV, W_UQ, W_KR, W_QR, W_O,
           **run_kwargs):
    in_maps = _host_prep(x, attention_mask, W_DKV, W_DQ, W_UK, W_UV, W_UQ,
                         W_KR, W_QR, W_O)
    mask_ones = bool(np.all(np.asarray(attention_mask) != 0))
    nc = _get_nc(mask_ones)
    res = run_bass_kernel_spmd(nc, in_maps, core_ids=list(range(8)),
                               **run_kwargs)
    out = np.zeros((B, L, D), np.float32)
    for c in range(8):
        out[c // HG] += res.results[c]["out"].astype(np.float32)
    if run_kwargs:
        _CACHE["last_results"] = res
    return out


# revision 12
# speedup vs baseline: 1.0312x; 1.0312x over previous
"""MLA attention distributed over 8 TRN2 NeuronCores.

Sharding: tensor-parallel over heads (4 head-groups) x data-parallel over
batch (2). Each core computes, for its (batch, head-group): the shared KV
compression, K/V up-projections for its 4 heads, a host-fused Q projection,
full attention for its 4 heads, and a partial output projection (its heads'
rows of W_O). Host gather sums the 4 partials per batch.

Key optimizations over the straightforward mapping:
  - All projection GEMMs (compression, K/V/Q up-projections, out-projection)
    run as hi-lo fp8 DoubleRow chains: each operand X is shipped/stored as a
    pair (hi, lo) with hi = e4m3(s*X), lo = e4m3(s*X - hi) -- the same byte
    count as bf16 but ~2.5x more accurate -- and X@W is computed as the three
    cross terms hi*hi + hi*lo + lo*hi, each a DoubleRow matmul contracting
    two 128-row planes per instruction.  That is 6 DR instructions per
    512-deep contraction instead of 4 bf16 instructions, at half the
    per-instruction cost: 25% less PE time than bf16 with better accuracy.
    The dropped lo*lo term is O(ulp^2).
  - The Q path is fused on the host: q = x @ (W_DQ @ [W_QR | W_UQ]) --
    one 640-wide GEMM instead of compression + up-projection.
  - Scores run on the PE in fp8e4m3 DoubleRow perf mode: the head's 128
    compressed dims are plane 0 and the shared rope dims are plane 1 of the
    doubled contraction (the rope block of the score matrix is
    head-independent, so it rides along as a second plane instead of a
    second matmul per head). q/k are stored at 8x scale to stay clear of the
    fp8 subnormal floor; the exp activation scale folds the 1/64 back out.
  - exp is batched two key-blocks per activation ([128,1024] from a 2-bank
    PSUM tile) to amortize the Activation engine's fixed access latency;
    with the all-ones attention mask the bias is a scalar 0. (A masked
    input falls back to per-key-block exp with a per-partition bias.)
  - The attention loop is software-pipelined at query-chunk granularity:
    the Q projection of chunk n+1, the out-projection (phase 5) of chunk
    n-1, and rope (on the idle Pool engine) all interleave into chunk n's
    score/AV emission, so the PE never waits on the Activation engine's
    exp round-trip. One accumulation chain per PSUM bank throughout (a
    start=True poisons the whole 2KB zero-region granule).
Attention probabilities and V run in bf16; softmax needs no max-subtraction
(scores are bounded ~|2|) and the denominator comes from a ones-column
appended to V.
"""

from collections import deque
from contextlib import ExitStack

import ml_dtypes
import numpy as np

import concourse.bacc as bacc
import concourse.mybir as mybir
import concourse.tile as tile
from concourse.bass_utils import run_bass_kernel_spmd
from concourse.masks import make_identity

B, L, D, H, DC, DH = 2, 2048, 2048, 16, 512, 128
HG = 4                 # head groups (tensor-parallel degree per batch)
HL = H // HG           # heads per core
HDL = HL * DH          # 512 head-dims per core
P = 128
N1 = 512               # matmul free-dim chunk
F32 = mybir.dt.float32
BF16 = mybir.dt.bfloat16
FP8 = mybir.dt.float8e4
F8NP = ml_dtypes.float8_e4m3
DR = mybir.MatmulPerfMode.DoubleRow
SCALE = 1.0 / float(np.sqrt(2 * DH))
FP8_PRE = 8.0          # q/k fp8 operand scale in kall/qall
XPRE = 16.0            # hi-lo pre-scale for activations (x)
WPRE = 400.0           # hi-lo pre-scale for weights
CTXPRE = 128.0         # hi-lo pre-scale for attention context
M1 = HDL + DH          # 640 fused output rows ([W_KR|W_DKV] / fused-Q)
MT1 = M1 // P          # 5
KT1 = D // P           # 16
NCH = L // N1          # 4 query/seq chunks
KT3 = DC // P          # 4
KB = L // P            # 16 key blocks
KBP = KB // 2          # 8 key-block pairs
DV = DH + 1            # value cols + denominator ones-column
NEG = -30000.0         # additive mask bias for masked-out keys
MULT = mybir.AluOpType.mult
SUB = mybir.AluOpType.subtract
# PSUM scale of the hi-lo projection chains and the derived copy-out scales
S1 = XPRE * WPRE            # ph1a / ph1b accumulate at 6400x
SCK = XPRE                  # on-chip c_kv pair is stored at 16x
S3 = SCK * WPRE             # ph3k / 3v accumulate at 6400x
S5 = CTXPRE * WPRE          # ph5 accumulates at 51200x


def build_nc(mask_ones=True):
    nc = bacc.Bacc(None, target_bir_lowering=False)

    # hi/lo pairs are interleaved innermost: [..., 2] with hi at index 0.
    xhl = nc.dram_tensor("xhl", [D, L * 2], FP8, kind="ExternalInput")
    w1a = nc.dram_tensor("w1a", [D, M1 * 2], FP8, kind="ExternalInput")
    w1b = nc.dram_tensor("w1b", [D, M1 * 2], FP8, kind="ExternalInput")
    wuk = nc.dram_tensor("wuk", [DC, HDL * 2], FP8, kind="ExternalInput")
    wuv = nc.dram_tensor("wuv", [DC, HDL * 2], FP8, kind="ExternalInput")
    wo = nc.dram_tensor("wo", [HDL, D * 2], FP8, kind="ExternalInput")
    cs_d = nc.dram_tensor("csT", [P, L], BF16, kind="ExternalInput")
    mask_d = nc.dram_tensor("maskb", [P, KB], F32, kind="ExternalInput")
    out_d = nc.dram_tensor("out", [L, D], BF16, kind="ExternalOutput")

    with tile.TileContext(nc) as tc, ExitStack() as es:
        # ---------- constants ----------
        p_const = es.enter_context(tc.tile_pool(name="const", bufs=1))
        bias_t = p_const.tile([P, KB], F32, name="bias_t")
        ident = p_const.tile([P, P], BF16, name="ident")
        make_identity(nc, ident[:])
        warm = p_const.tile([P, 1], F32, name="warm")
        nc.scalar.activation(warm[:], bias_t[:, 0:1],
                             mybir.ActivationFunctionType.Exp)


        # ---------- right-side residents (live until end) ----------
        es_tab = ExitStack()
        p_tab = es_tab.enter_context(tc.tile_pool(name="tabp", bufs=1,
                                                  side="right"))
        cos_t = p_tab.tile([DH // 2, L], BF16, name="cos_t")
        sin_t = p_tab.tile([DH // 2, L], BF16, name="sin_t")

        es_xr = ExitStack()
        p_xr = es_xr.enter_context(tc.tile_pool(name="xrp", bufs=1,
                                                side="right"))
        xr_t = p_xr.tile([P, L], BF16, name="xrT")
        xrk_t = xrq_t = xr_t   # xrk is dead before xrq is first written

        es_w1b = ExitStack()
        p_w1b = es_w1b.enter_context(tc.tile_pool(name="w1bp", bufs=1,
                                                  side="right"))
        es_rope = ExitStack()
        p_rope = es_rope.enter_context(tc.tile_pool(name="ropep", bufs=1,
                                                    side="right"))

        # ---------- long-lived left-side pools (bottom of stack) ----------
        # attention operands: planes 0..3 = per-head compressed dims,
        # plane 4 = shared rope dims (packed fp8 for DoubleRow)
        es_att = ExitStack()
        p_att = es_att.enter_context(tc.tile_pool(name="attp", bufs=1))
        kall = p_att.tile([P, HL + 1, L], FP8, name="kall")
        qall = p_att.tile([P, HL + 1, L], FP8, name="qall")
        vaug_t = [p_att.tile([P, HL * DV], BF16, tag=f"v{i}", name=f"v{i}")
                  for i in range(KB)]

        # ---------- transient pools for phases 1a/3 ----------
        # wukv/ckv feed the 3v filler chains inside the attention loop, so
        # they sit below x/w1a and stay open until the end
        es_ps13 = ExitStack()
        p_ps13 = es_ps13.enter_context(tc.tile_pool(name="ps13", bufs=3,
                                                    space="PSUM"))
        es_wukv = ExitStack()
        p_wuk = es_wukv.enter_context(tc.tile_pool(name="wukp", bufs=1))
        p_wuv = es_wukv.enter_context(tc.tile_pool(name="wuvp", bufs=1))
        es_ckv = ExitStack()
        p_ckv = es_ckv.enter_context(tc.tile_pool(name="ckvp", bufs=1))
        ckv2 = p_ckv.tile([P, KT3, L, 2], FP8, name="ckv2")
        es_x = ExitStack()
        p_x = es_x.enter_context(tc.tile_pool(name="xp", bufs=1))
        es_w1a = ExitStack()
        p_w1a = es_w1a.enter_context(tc.tile_pool(name="w1ap", bufs=1))

        # 3-term hi-lo DoubleRow chain: stat/mov are [P, KT, cols, 2] tiles
        # (hi at [..., 0]); each kt-pair contributes hi*hi, hi*lo, lo*hi.
        def hl_chain(ps, stat, scols, mov, mcols, nkt):
            first = True
            for kt in range(0, nkt, 2):
                for (a, b) in ((0, 0), (0, 1), (1, 0)):
                    last = kt == nkt - 2 and (a, b) == (1, 0)
                    nc.tensor.matmul(ps, stat[:, kt:kt + 2, scols, a],
                                     mov[:, kt:kt + 2, mcols, b],
                                     start=first, stop=last, perf_mode=DR)
                    first = False

        # hi-lo split of a PSUM chain result into an interleaved fp8 pair:
        # hi = e4m3(ps*sc), lo = e4m3(ps*sc - hi)
        def hl_split(eng, dst_hi, dst_lo, ps, sc):
            eng.tensor_scalar_mul(dst_hi, ps, sc)
            eng.scalar_tensor_tensor(dst_lo, ps, sc, dst_hi, MULT, SUB)

        # rope in transposed layout: even rows 0:64 / odd rows 64:128 of the
        # pre-roped projection (host permuted the weight columns). Runs on
        # the Pool (gpsimd) engine, per sequence chunk, writing fp8 planes.
        def rope_chunk(src_t, dst_pl, pfx, ch):
            cs = slice(ch * N1, (ch + 1) * N1)
            eng = nc.gpsimd
            xo = p_rope.tile([64, N1], BF16, tag="rxo", name=f"{pfx}xo{ch}")
            eng.dma_start(xo[:], src_t[64:P, cs])
            t1 = p_rope.tile([64, N1], BF16, tag="rt1", name=f"{pfx}t1{ch}")
            t2 = p_rope.tile([64, N1], BF16, tag="rt2", name=f"{pfx}t2{ch}")
            h2 = p_rope.tile([64, N1], FP8, tag="rh2", name=f"{pfx}h2{ch}")
            xe = src_t[0:64, cs]
            cc, ss = cos_t[:, cs], sin_t[:, cs]
            eng.tensor_tensor(t1[:], xe, cc, MULT)
            eng.tensor_tensor(t2[:], xo[:], ss, MULT)
            eng.tensor_tensor(dst_pl[0:64, cs], t1[:], t2[:], SUB)
            t3 = p_rope.tile([64, N1], BF16, tag="rt1", name=f"{pfx}t3{ch}")
            t4 = p_rope.tile([64, N1], BF16, tag="rt2", name=f"{pfx}t4{ch}")
            eng.tensor_tensor(t3[:], xe, ss, MULT)
            eng.tensor_tensor(t4[:], xo[:], cc, MULT)
            eng.tensor_tensor(h2[:], t3[:], t4[:], mybir.AluOpType.add)
            eng.dma_start(dst_pl[64:P, cs], h2[:])

        # ---------- DMA order on the sync queue: strict priority ----------
        # One big strided transfer per logical tensor: HWDGE descriptor
        # generation costs ~630ns per dma_start, so few large calls beat
        # many tile-sized ones. Order: w1a rope-cols, x chunk0, w1a rest,
        # x chunks 1-3, wuk, wuv (w1b/wo queued after the 1a loop).
        xT_r = xhl.rearrange("(k p) (l t) -> p k l t", p=P, t=2)
        w1a_r = w1a.rearrange("(k p) (m t) -> p k m t", p=P, t=2)
        xb2 = p_x.tile([P, KT1, L, 2], FP8, name="xb2")
        w1a2 = p_w1a.tile([P, KT1, M1, 2], FP8, name="w1a2")
        for k0, k1 in ((0, 2), (2, 4), (4, 8), (8, 12)):
            nc.sync.dma_start(w1a2[:, k0:k1, 0:P, :], w1a_r[:, k0:k1, 0:P, :])
            nc.sync.dma_start(xb2[:, k0:k1, 0:N1, :], xT_r[:, k0:k1, 0:N1, :])
        nc.sync.dma_start(w1a2[:, 12:KT1, 0:P, :], w1a_r[:, 12:KT1, 0:P, :])
        nc.sync.dma_start(w1a2[:, 0:8, P:2 * P, :], w1a_r[:, 0:8, P:2 * P, :])
        nc.sync.dma_start(xb2[:, 12:KT1, 0:N1, :], xT_r[:, 12:KT1, 0:N1, :])
        nc.sync.dma_start(w1a2[:, 8:KT1, P:2 * P, :],
                          w1a_r[:, 8:KT1, P:2 * P, :])
        nc.sync.dma_start(w1a2[:, :, 2 * P:3 * P, :],
                          w1a_r[:, :, 2 * P:3 * P, :])
        nc.sync.dma_start(cos_t[:], cs_d[0:DH // 2, :])
        nc.sync.dma_start(w1a2[:, :, 3 * P:4 * P, :], w1a_r[:, :, 3 * P:4 * P, :])
        nc.sync.dma_start(bias_t[:], mask_d[:])
        nc.sync.dma_start(xb2[:, :, N1:2 * N1, :], xT_r[:, :, N1:2 * N1, :])
        nc.sync.dma_start(w1a2[:, :, 4 * P:M1, :], w1a_r[:, :, 4 * P:M1, :])
        nc.sync.dma_start(sin_t[:], cs_d[DH // 2:P, :])
        for nci in range(2, NCH):
            nc.sync.dma_start(xb2[:, :, nci * N1:(nci + 1) * N1, :],
                              xT_r[:, :, nci * N1:(nci + 1) * N1, :])
        wuk2 = p_wuk.tile([P, KT3, HDL, 2], FP8, name="wuk2")
        nc.sync.dma_start(wuk2[:],
                          wuk.rearrange("(k p) (m t) -> p k m t", p=P, t=2))
        wuv2 = p_wuv.tile([P, KT3, HDL, 2], FP8, name="wuv2")
        nc.sync.dma_start(wuv2[:],
                          wuv.rearrange("(k p) (m t) -> p k m t", p=P, t=2))

        # ---------- phase 1a: [xrkT | c_kvT] = [Wkr | Wdkv].T @ x.T ----
        # mt order puts the rope row-block first so each chunk's rope can
        # run on Pool while the PE continues with the c_kv rows.
        MTO = [MT1 - 1] + list(range(MT1 - 1))
        for nci in range(NCH):
            cs = slice(nci * N1, (nci + 1) * N1)
            for sl, mt in enumerate(MTO):
                ps = p_ps13.tile([P, N1], F32, tag="g", name=f"ps1a_{nci}_{mt}")
                hl_chain(ps[:], w1a2, slice(sl * P, (sl + 1) * P),
                         xb2, cs, KT1)
                if mt == MT1 - 1:
                    nc.vector.tensor_scalar_mul(xrk_t[:, cs], ps[:],
                                                FP8_PRE / S1)
                    rope_chunk(xrk_t, kall[:, HL, :], "k", nci)
                else:
                    hl_split(nc.vector, ckv2[:, mt, cs, 0], ckv2[:, mt, cs, 1],
                             ps[:], SCK / S1)
        # w1b prefetch queued behind the x stream
        w1b2 = p_w1b.tile([P, KT1, M1, 2], FP8, name="w1b2")
        nc.sync.dma_start(w1b2[:],
                          w1b.rearrange("(k p) (m t) -> p k m t", p=P, t=2))
        es_w1a.close()

        # ---------- phase 3k: k_cT = Wuk_hg.T @ c_kvT -> fp8 plane ----
        for nci in range(NCH):
            cs = slice(nci * N1, (nci + 1) * N1)
            for h in range(HL):
                ps = p_ps13.tile([P, N1], F32, tag="g", name=f"ps3k_{nci}_{h}")
                hl_chain(ps[:], wuk2, slice(h * P, (h + 1) * P),
                         ckv2, cs, KT3)
                nc.vector.tensor_scalar_mul(kall[:, h, cs], ps[:],
                                            FP8_PRE / S3)

        # ---------- phase 1b: [xrqT | q_cT] = fusedWq.T @ x.T ----------
        qdest = [qall[:, h, :] for h in range(HL)] + [xrq_t[:]]
        for nci in range(NCH):
            cs = slice(nci * N1, (nci + 1) * N1)
            for sl, mt in enumerate(MTO):
                ps = p_ps13.tile([P, N1], F32, tag="g", name=f"ps1b_{nci}_{mt}")
                hl_chain(ps[:], w1b2, slice(sl * P, (sl + 1) * P),
                         xb2, cs, KT1)
                nc.vector.tensor_scalar_mul(qdest[mt][:, cs], ps[:],
                                            FP8_PRE / S1)
                if mt == MT1 - 1:
                    rope_chunk(xrq_t, qall[:, HL, :], "q", nci)
        es_ps13.close()
        es_x.close()

        # ---------- pools for the fused 1b + attention + out-proj --------
        es_wo = ExitStack()
        p_wo = es_wo.enter_context(tc.tile_pool(name="wop", bufs=1,
                                                side="right"))
        wo2 = p_wo.tile([P, HL, D, 2], FP8, name="wo2")
        nc.sync.dma_start(wo2[:],
                          wo.rearrange("(k p) (d t) -> p k d t", p=P, t=2))
        es_ctx = ExitStack()
        p_ctx = es_ctx.enter_context(tc.tile_pool(name="ctxp", bufs=1))
        ctx2 = p_ctx.tile([P, HL, L, 2], FP8, name="ctx2")
        es_p4 = ExitStack()
        p_sc = es_p4.enter_context(tc.tile_pool(name="scp", bufs=2,
                                                space="PSUM"))
        p_av = es_p4.enter_context(tc.tile_pool(name="avp", bufs=2,
                                                space="PSUM"))
        # shared ring for ph5 chains and (bitcast) transpose targets; they
        # never interleave inside one bank
        p_ch = es_p4.enter_context(tc.tile_pool(name="chp", bufs=2,
                                                space="PSUM"))
        p_e = es_p4.enter_context(tc.tile_pool(name="expp", bufs=16))
        p_sm = es_p4.enter_context(tc.tile_pool(name="smallp", bufs=8))
        p_st = es_p4.enter_context(tc.tile_pool(name="stagep", bufs=4))

        escale = float(SCALE / (FP8_PRE * FP8_PRE))

        # -- phase-5 blocks of query chunk n-1, interleaved as fillers --
        # stage a full [P, D] row-block in bf16, one out-DMA per mt (4 calls
        # per query chunk instead of 16 — HWDGE desc-gen is 625ns per call)
        ph5_pending = deque()
        ph5_stage = {}

        def emit_ph5_block(mt, nci, ps):
            hl_chain(ps, ctx2, slice(mt * P, (mt + 1) * P),
                     wo2, slice(nci * N1, (nci + 1) * N1), HL)
            if nci == 0:
                ph5_stage[mt] = p_st.tile([P, D], BF16, tag="stage",
                                          name=f"st_{mt}")
            stg = ph5_stage[mt]
            nc.vector.tensor_scalar_mul(
                stg[:, nci * N1:(nci + 1) * N1], ps, 1.0 / S5)
            if nci == NCH - 1:
                nc.sync.dma_start(out_d[mt * P:(mt + 1) * P, :],
                                  ph5_stage.pop(mt)[:])

        # 3v chains (v = c_kv @ Wuv, bf16 + ones col) run as qch0-hs0
        # fillers: AV first touches vaug at hs1, scores don't need it
        def gen_3v(mt):
            ps = p_ch.tile([P, N1], F32, tag="ch", name=f"psv_{mt}")
            hl_chain(ps[:], ckv2, slice(mt * P, (mt + 1) * P),
                     wuv2, slice(0, HDL), KT3)
            va = vaug_t[mt].rearrange("p (h c) -> p h c", c=DV)
            nc.vector.tensor_scalar_mul(
                va[:, :, 0:DH], ps.rearrange("p (h c) -> p h c", c=DH),
                1.0 / S3)
            nc.vector.memset(va[:, :, DH:DV], 1.0)

        v3_pending = deque(range(KB))

        def filler():
            if v3_pending:
                gen_3v(v3_pending.popleft())
            elif ph5_pending:
                mt, nci = ph5_pending.popleft()
                ps = p_ch.tile([P, N1], F32, tag="ch",
                               name=f"ps5_{mt}_{nci}")
                emit_ph5_block(mt, nci, ps[:])


        # ---------- fused attention + out-projection loop ----------
        et_t = {}              # (h, kbp) -> exp tile, two key-blocks wide
        av_t = {}              # (h, qc) -> psum chain tile [P, N1] (1 bank)

        def av_chain_seg(hp, qc, kbp, half, first, last):
            kb = 2 * kbp + half
            nc.tensor.matmul(
                av_t[(hp, qc)][:, 0:DV],
                et_t[(hp, kbp)][:, half, qc * P:(qc + 1) * P],
                vaug_t[kb][:, hp * DV:(hp + 1) * DV],
                start=first, stop=last)

        def norm(qch, hp, qc):
            pc = av_t.pop((hp, qc))
            rc = p_sm.tile([P, 1], F32, tag="recip", name=f"rc_{qch}_{hp}_{qc}")
            nc.vector.reciprocal(rc[:], pc[:, DH:DV])
            cn = p_sm.tile([P, DH], BF16, tag="cn", name=f"cn_{qch}_{hp}_{qc}")
            nc.vector.tensor_scalar_mul(cn[:], pc[:, 0:DH], rc[:])
            et_t[("cn", hp, qc)] = cn

        for qch in range(NCH):
            qs = slice(qch * N1, (qch + 1) * N1)
            for hs in range(HL + 1):
                # transposes for head hs-2 (deferred so the DVE norm of that
                # head has certainly drained); bitcast [P,N1]f32 chain slots
                # to bf16 for the [P,P] transpose target
                if hs >= 2 or (qch > 0 and hs == 0):
                    hp2 = hs - 2 if hs >= 2 else HL - 1
                    qq = qch if hs >= 2 else qch - 1
                    for qc in range(4):
                        q0 = qq * 4 + qc
                        cn = et_t.pop(("cn", hp2, qc))
                        pt = p_ch.tile([P, N1], F32, tag="ch",
                                       name=f"tp_{qq}_{hp2}_{q0}")
                        ptb = pt[:].bitcast(BF16)[:, 0:P]
                        nc.tensor.transpose(ptb, cn[:], ident[:])
                        hl_split(nc.vector,
                                 ctx2[:, hp2, q0 * P:(q0 + 1) * P, 0],
                                 ctx2[:, hp2, q0 * P:(q0 + 1) * P, 1],
                                 ptb, CTXPRE)
                hp = hs - 1
                if hs > 0:
                    # one PSUM bank per accumulation chain: a start=True in a
                    # bank poisons the whole 2KB zero-region granule, so two
                    # interleaved chains must never share a bank
                    for qc in range(2):
                        av_t[(hp, qc)] = p_av.tile(
                            [P, N1], F32, tag="av", name=f"av_{qch}_{hp}_{qc}")
                # sweep 1: scores/exp of head hs + AV chains qc0/qc1 of
                # hs-1. The AV consumption order is rotated by +2 key-block
                # pairs so the chain starts on already-drained exp tiles
                # (PSUM accumulation is commutative); SC is emitted after
                # the AV/filler work so a psum-slot wait never idles the PE.
                for i in range(KBP):
                    kbp = i
                    if hs > 0:
                        if i % 4 == 3 and hs < HL:
                            filler()
                        for half in range(2):
                            av_chain_seg(hp, 0, kbp, half, i == 0 and half == 0,
                                         i == KBP - 1 and half == 1)
                            av_chain_seg(hp, 1, kbp, half, i == 0 and half == 0,
                                         i == KBP - 1 and half == 1)
                    elif qch == 0:
                        filler()
                        filler()
                    else:
                        filler()
                    if hs < HL:
                        sc = p_sc.tile([P, 2, N1], F32, tag="sc",
                                       name=f"sc_{qch}_{hs}_{i}")
                        et = p_e.tile([P, 2, N1], BF16, tag="expT",
                                      name=f"et_{qch}_{hs}_{i}")
                        for half in range(2):
                            kb = 2 * i + half
                            nc.tensor.matmul(
                                sc[:, half, :],
                                kall[:, hs::(HL - hs), kb * P:(kb + 1) * P],
                                qall[:, hs::(HL - hs), qs],
                                start=True, stop=True, perf_mode=DR)
                        if mask_ones:
                            nc.scalar.activation(
                                et[:], sc[:],
                                mybir.ActivationFunctionType.Exp,
                                scale=escale)
                        else:
                            for half in range(2):
                                kb = 2 * i + half
                                nc.scalar.activation(
                                    et[:, half, :], sc[:, half, :],
                                    mybir.ActivationFunctionType.Exp,
                                    bias=bias_t[:, kb:kb + 1], scale=escale)
                        et_t[(hs, i)] = et
                # sweep 2: AV chains qc2/qc3 of head hs-1 + norms
                if hs > 0:
                    norm(qch, hp, 0)
                    norm(qch, hp, 1)
                    for qc in range(2, 4):
                        av_t[(hp, qc)] = p_av.tile(
                            [P, N1], F32, tag="av", name=f"av_{qch}_{hp}_{qc}")
                    for i in range(KBP):
                        for half in range(2):
                            av_chain_seg(hp, 2, i, half, i == 0 and half == 0,
                                         i == KBP - 1 and half == 1)
                            av_chain_seg(hp, 3, i, half, i == 0 and half == 0,
                                         i == KBP - 1 and half == 1)
                        if i % 4 == 1:
                            filler()
                    for kbp in range(KBP):
                        et_t.pop((hp, kbp))
                    norm(qch, hp, 2)
                    norm(qch, hp, 3)
            ph5_pending.extend((qch * 4 + mt, nci)
                               for mt in range(4) for nci in range(NCH))


        # flush: transposes of the last head + phase 5 for the last chunk;
        # alternate two psum rings (sc is free now) so chains double-buffer
        for qc in range(4):
            q0 = (NCH - 1) * 4 + qc
            cn = et_t.pop(("cn", HL - 1, qc))
            pt = p_ch.tile([P, N1], F32, tag="ch", name=f"tpf_{q0}")
            ptb = pt[:].bitcast(BF16)[:, 0:P]
            nc.tensor.transpose(ptb, cn[:], ident[:])
            hl_split(nc.vector,
                     ctx2[:, HL - 1, q0 * P:(q0 + 1) * P, 0],
                     ctx2[:, HL - 1, q0 * P:(q0 + 1) * P, 1],
                     ptb, CTXPRE)
        fi = 0
        while ph5_pending:
            mt, nci = ph5_pending.popleft()
            if fi % 2 == 0:
                ps = p_sc.tile([P, 2, N1], F32, tag="sc",
                               name=f"ps5f_{fi}")[:, 0, :]
            else:
                ps = p_ch.tile([P, N1], F32, tag="ch", name=f"ps5f_{fi}")[:]
            fi += 1
            emit_ph5_block(mt, nci, ps)

        es_p4.close()
        es_ctx.close()
        es_ckv.close()
        es_wukv.close()
        es_att.close()
        es_wo.close()
        es_rope.close()
        es_w1b.close()
        es_xr.close()
        es_tab.close()

    nc.compile()
    return nc


_CACHE = {}


def _get_nc(mask_ones=True):
    key = ("nc", mask_ones)
    if key not in _CACHE:
        _CACHE[key] = build_nc(mask_ones)
    return _CACHE[key]


def _split8(a, pre):
    """hi-lo e4m3 pair of pre*a, interleaved innermost: [d0, d1*2]."""
    s = np.asarray(a, np.float32) * np.float32(pre)
    hi = s.astype(F8NP)
    lo = (s - hi.astype(np.float32)).astype(F8NP)
    return np.ascontiguousarray(np.stack([hi, lo], axis=-1).reshape(
        a.shape[0], a.shape[1] * 2))


def _host_prep(x, attention_mask, W_DKV, W_DQ, W_UK, W_UV, W_UQ, W_KR, W_QR,
               W_O):
    f = np.float32
    bf = ml_dtypes.bfloat16
    x = np.asarray(x, f)
    attention_mask = np.asarray(attention_mask)
    W_DKV, W_DQ = np.asarray(W_DKV, f), np.asarray(W_DQ, f)
    W_UK, W_UV, W_UQ = np.asarray(W_UK, f), np.asarray(W_UV, f), np.asarray(W_UQ, f)
    W_KR, W_QR, W_O = np.asarray(W_KR, f), np.asarray(W_QR, f), np.asarray(W_O, f)

    perm = np.concatenate([np.arange(0, DH, 2), np.arange(1, DH, 2)])
    # column layout [rope | dkv]: the rope block is computed first on-chip
    w1a = _split8(np.concatenate([W_KR[:, perm], W_DKV], axis=1), WPRE)
    xhls = [_split8(x[b].T, XPRE) for b in range(B)]

    inv = 1.0 / (10000.0 ** (np.arange(0, DH, 2, dtype=f) / DH))
    freqs = np.arange(L, dtype=f)[:, None] * inv[None, :]
    rope = np.concatenate([np.sin(freqs), np.cos(freqs)], axis=-1).astype(f)
    csT = np.ascontiguousarray(np.concatenate(
        [rope[:, 1::2].T, rope[:, 0::2].T], axis=0).astype(bf))

    maskbs = []
    for b in range(B):
        bias = np.where(attention_mask[b] == 0, f(NEG), f(0.0)).astype(f)
        maskbs.append(np.ascontiguousarray(bias.reshape(KB, P).T))

    w1bs, wuks, wuvs, wos = [], [], [], []
    for hg in range(HG):
        cols = slice(hg * HDL, (hg + 1) * HDL)
        wq = W_DQ @ np.concatenate([W_QR[:, perm], W_UQ[:, cols]], axis=1)
        w1bs.append(_split8(wq, WPRE))
        wuks.append(_split8(W_UK[:, cols], WPRE))
        wuvs.append(_split8(W_UV[:, cols], WPRE))
        wos.append(_split8(W_O[hg * HDL:(hg + 1) * HDL, :], WPRE))

    in_maps = []
    for c in range(8):
        b, hg = c // HG, c % HG
        in_maps.append({
            "xhl": xhls[b],
            "w1a": w1a,
            "w1b": w1bs[hg],
            "wuk": wuks[hg],
            "wuv": wuvs[hg],
            "wo": wos[hg],
            "csT": csT,
            "maskb": maskbs[b],
        })
    return in_maps


def kernel(x, attention_mask, W_DKV, W_DQ, W_UK, W_UV, W_UQ, W_KR, W_QR, W_O,
           **run_kwargs):
    in_maps = _host_prep(x, attention_mask, W_DKV, W_DQ, W_UK, W_UV, W_UQ,
                         W_KR, W_QR, W_O)
    mask_ones = bool(np.all(np.asarray(attention_mask) != 0))
    nc = _get_nc(mask_ones)
    res = run_bass_kernel_spmd(nc, in_maps, core_ids=list(range(8)),
                               **run_kwargs)
    out = np.zeros((B, L, D), np.float32)
    for c in range(8):
        out[c // HG] += res.results[c]["out"].astype(np.float32)
    if run_kwargs:
        _CACHE["last_results"] = res
    return out


# revision 13
# speedup vs baseline: 1.0694x; 1.0371x over previous
"""MLA attention distributed over 8 TRN2 NeuronCores.

Sharding: tensor-parallel over heads (4 head-groups) x data-parallel over
batch (2). Each core computes, for its (batch, head-group): the shared KV
compression, K/V up-projections for its 4 heads, a host-fused Q projection,
full attention for its 4 heads, and a partial output projection (its heads'
rows of W_O). Host gather sums the 4 partials per batch.

Key optimizations over the straightforward mapping:
  - All projection GEMMs (compression, K/V/Q up-projections, out-projection)
    run as hi-lo fp8 DoubleRow chains: each operand X is shipped/stored as a
    pair (hi, lo) with hi = e4m3(s*X), lo = e4m3(s*X - hi) -- the same byte
    count as bf16 but ~2.5x more accurate -- and X@W is computed as the three
    cross terms hi*hi + hi*lo + lo*hi, each a DoubleRow matmul contracting
    two 128-row planes per instruction.  That is 6 DR instructions per
    512-deep contraction instead of 4 bf16 instructions, at half the
    per-instruction cost: 25% less PE time than bf16 with better accuracy.
    The dropped lo*lo term is O(ulp^2).
  - The Q path is fused on the host: q = x @ (W_DQ @ [W_QR | W_UQ]) --
    one 640-wide GEMM instead of compression + up-projection.
  - Scores run on the PE in fp8e4m3 DoubleRow perf mode: the head's 128
    compressed dims are plane 0 and the shared rope dims are plane 1 of the
    doubled contraction (the rope block of the score matrix is
    head-independent, so it rides along as a second plane instead of a
    second matmul per head). q/k are stored at 8x scale to stay clear of the
    fp8 subnormal floor; the exp activation scale folds the 1/64 back out.
  - exp is batched two key-blocks per activation ([128,1024] from a 2-bank
    PSUM tile) to amortize the Activation engine's fixed access latency;
    with the all-ones attention mask the bias is a scalar 0. (A masked
    input falls back to per-key-block exp with a per-partition bias.)
  - The attention loop is software-pipelined at query-chunk granularity:
    the Q projection of chunk n+1, the out-projection (phase 5) of chunk
    n-1, and rope (on the idle Pool engine) all interleave into chunk n's
    score/AV emission, so the PE never waits on the Activation engine's
    exp round-trip. One accumulation chain per PSUM bank throughout (a
    start=True poisons the whole 2KB zero-region granule).
Attention probabilities and V run in bf16; softmax needs no max-subtraction
(scores are bounded ~|2|) and the denominator comes from a ones-column
appended to V.
"""

from collections import deque
from contextlib import ExitStack

import ml_dtypes
import numpy as np

import concourse.bacc as bacc
import concourse.mybir as mybir
import concourse.tile as tile
from concourse.bass_utils import run_bass_kernel_spmd
from concourse.masks import make_identity

B, L, D, H, DC, DH = 2, 2048, 2048, 16, 512, 128
HG = 4                 # head groups (tensor-parallel degree per batch)
HL = H // HG           # heads per core
HDL = HL * DH          # 512 head-dims per core
P = 128
N1 = 512               # matmul free-dim chunk
F32 = mybir.dt.float32
BF16 = mybir.dt.bfloat16
FP8 = mybir.dt.float8e4
F8NP = ml_dtypes.float8_e4m3
DR = mybir.MatmulPerfMode.DoubleRow
SCALE = 1.0 / float(np.sqrt(2 * DH))
FP8_PRE = 8.0          # q/k fp8 operand scale in kall/qall
XPRE = 16.0            # hi-lo pre-scale for activations (x)
WPRE = 400.0           # hi-lo pre-scale for weights
CTXPRE = 128.0         # hi-lo pre-scale for attention context
M1 = HDL + DH          # 640 fused output rows ([W_KR|W_DKV] / fused-Q)
MT1 = M1 // P          # 5
KT1 = D // P           # 16
NCH = L // N1          # 4 query/seq chunks
KT3 = DC // P          # 4
KB = L // P            # 16 key blocks
KBP = KB // 2          # 8 key-block pairs
DV = DH + 1            # value cols + denominator ones-column
NEG = -30000.0         # additive mask bias for masked-out keys
MULT = mybir.AluOpType.mult
SUB = mybir.AluOpType.subtract
# PSUM scale of the hi-lo projection chains and the derived copy-out scales
S1 = XPRE * WPRE            # ph1a / ph1b accumulate at 6400x
SCK = XPRE                  # on-chip c_kv pair is stored at 16x
S3 = SCK * WPRE             # ph3k / 3v accumulate at 6400x
S5 = CTXPRE * WPRE          # ph5 accumulates at 51200x


def build_nc(mask_ones=True):
    nc = bacc.Bacc(None, target_bir_lowering=False)

    # hi/lo pairs are interleaved innermost: [..., 2] with hi at index 0.
    xhl = nc.dram_tensor("xhl", [D, L * 2], FP8, kind="ExternalInput")
    w1a = nc.dram_tensor("w1a", [D, M1 * 2], FP8, kind="ExternalInput")
    w1b = nc.dram_tensor("w1b", [D, M1 * 2], FP8, kind="ExternalInput")
    wuk = nc.dram_tensor("wuk", [DC, HDL * 2], FP8, kind="ExternalInput")
    wuv = nc.dram_tensor("wuv", [DC, HDL * 2], FP8, kind="ExternalInput")
    wo = nc.dram_tensor("wo", [HDL, D * 2], FP8, kind="ExternalInput")
    cs_d = nc.dram_tensor("csT", [P, L], BF16, kind="ExternalInput")
    mask_d = nc.dram_tensor("maskb", [P, KB], F32, kind="ExternalInput")
    out_d = nc.dram_tensor("out", [L, D], BF16, kind="ExternalOutput")

    with tile.TileContext(nc) as tc, ExitStack() as es:
        # ---------- constants ----------
        p_const = es.enter_context(tc.tile_pool(name="const", bufs=1))
        bias_t = p_const.tile([P, KB], F32, name="bias_t")
        ident = p_const.tile([P, P], BF16, name="ident")
        make_identity(nc, ident[:])
        warm = p_const.tile([P, 1], F32, name="warm")
        nc.scalar.activation(warm[:], bias_t[:, 0:1],
                             mybir.ActivationFunctionType.Exp)


        # ---------- right-side residents (live until end) ----------
        es_tab = ExitStack()
        p_tab = es_tab.enter_context(tc.tile_pool(name="tabp", bufs=1,
                                                  side="right"))
        cos_t = p_tab.tile([DH // 2, L], BF16, name="cos_t")
        sin_t = p_tab.tile([DH // 2, L], BF16, name="sin_t")

        es_xr = ExitStack()
        p_xr = es_xr.enter_context(tc.tile_pool(name="xrp", bufs=1,
                                                side="right"))
        xr_t = p_xr.tile([P, L], BF16, name="xrT")
        xrk_t = xrq_t = xr_t   # xrk is dead before xrq is first written

        es_w1b = ExitStack()
        p_w1b = es_w1b.enter_context(tc.tile_pool(name="w1bp", bufs=1,
                                                  side="right"))
        es_rope = ExitStack()
        p_rope = es_rope.enter_context(tc.tile_pool(name="ropep", bufs=1,
                                                    side="right"))

        # ---------- long-lived left-side pools (bottom of stack) ----------
        # attention operands: planes 0..3 = per-head compressed dims,
        # plane 4 = shared rope dims (packed fp8 for DoubleRow)
        es_att = ExitStack()
        p_att = es_att.enter_context(tc.tile_pool(name="attp", bufs=1))
        kall = p_att.tile([P, HL + 1, L], FP8, name="kall")
        qall = p_att.tile([P, HL + 1, L], FP8, name="qall")
        vaug_t = [p_att.tile([P, HL * DV], BF16, tag=f"v{i}", name=f"v{i}")
                  for i in range(KB)]

        # ---------- transient pools for phases 1a/3 ----------
        # wukv/ckv feed the 3v filler chains inside the attention loop, so
        # they sit below x/w1a and stay open until the end
        es_ps13 = ExitStack()
        p_ps13 = es_ps13.enter_context(tc.tile_pool(name="ps13", bufs=3,
                                                    space="PSUM"))
        es_wukv = ExitStack()
        p_wuk = es_wukv.enter_context(tc.tile_pool(name="wukp", bufs=1))
        p_wuv = es_wukv.enter_context(tc.tile_pool(name="wuvp", bufs=1))
        es_ckv = ExitStack()
        p_ckv = es_ckv.enter_context(tc.tile_pool(name="ckvp", bufs=1))
        ckv2 = p_ckv.tile([P, KT3, L, 2], FP8, name="ckv2")
        es_x = ExitStack()
        p_x = es_x.enter_context(tc.tile_pool(name="xp", bufs=1))
        es_w1a = ExitStack()
        p_w1a = es_w1a.enter_context(tc.tile_pool(name="w1ap", bufs=1))

        # 3-term hi-lo DoubleRow chain: stat/mov are [P, KT, cols, 2] tiles
        # (hi at [..., 0]); each kt-pair contributes hi*hi, hi*lo, lo*hi.
        def hl_chain(ps, stat, scols, mov, mcols, nkt):
            first = True
            for kt in range(0, nkt, 2):
                for (a, b) in ((0, 0), (0, 1), (1, 0)):
                    last = kt == nkt - 2 and (a, b) == (1, 0)
                    nc.tensor.matmul(ps, stat[:, kt:kt + 2, scols, a],
                                     mov[:, kt:kt + 2, mcols, b],
                                     start=first, stop=last, perf_mode=DR)
                    first = False

        # hi-lo split of a PSUM chain result into an interleaved fp8 pair:
        # hi = e4m3(ps*sc), lo = e4m3(ps*sc - hi)
        def hl_split(eng, dst_hi, dst_lo, ps, sc):
            eng.tensor_scalar_mul(dst_hi, ps, sc)
            eng.scalar_tensor_tensor(dst_lo, ps, sc, dst_hi, MULT, SUB)

        # rope in transposed layout: even rows 0:64 / odd rows 64:128 of the
        # pre-roped projection (host permuted the weight columns). Runs on
        # the Pool (gpsimd) engine, per sequence chunk, writing fp8 planes.
        def rope_chunk(src_t, dst_pl, pfx, ch):
            cs = slice(ch * N1, (ch + 1) * N1)
            eng = nc.gpsimd
            xo = p_rope.tile([64, N1], BF16, tag="rxo", name=f"{pfx}xo{ch}")
            eng.dma_start(xo[:], src_t[64:P, cs])
            t1 = p_rope.tile([64, N1], BF16, tag="rt1", name=f"{pfx}t1{ch}")
            t2 = p_rope.tile([64, N1], BF16, tag="rt2", name=f"{pfx}t2{ch}")
            h2 = p_rope.tile([64, N1], FP8, tag="rh2", name=f"{pfx}h2{ch}")
            xe = src_t[0:64, cs]
            cc, ss = cos_t[:, cs], sin_t[:, cs]
            eng.tensor_tensor(t1[:], xe, cc, MULT)
            eng.tensor_tensor(t2[:], xo[:], ss, MULT)
            eng.tensor_tensor(dst_pl[0:64, cs], t1[:], t2[:], SUB)
            t3 = p_rope.tile([64, N1], BF16, tag="rt1", name=f"{pfx}t3{ch}")
            t4 = p_rope.tile([64, N1], BF16, tag="rt2", name=f"{pfx}t4{ch}")
            eng.tensor_tensor(t3[:], xe, ss, MULT)
            eng.tensor_tensor(t4[:], xo[:], cc, MULT)
            eng.tensor_tensor(h2[:], t3[:], t4[:], mybir.AluOpType.add)
            eng.dma_start(dst_pl[64:P, cs], h2[:])

        # ---------- DMA order on the sync queue: strict priority ----------
        # One big strided transfer per logical tensor: HWDGE descriptor
        # generation costs ~630ns per dma_start, so few large calls beat
        # many tile-sized ones. Order: w1a rope-cols, x chunk0, w1a rest,
        # x chunks 1-3, wuk, wuv (w1b/wo queued after the 1a loop).
        xT_r = xhl.rearrange("(k p) (l t) -> p k l t", p=P, t=2)
        w1a_r = w1a.rearrange("(k p) (m t) -> p k m t", p=P, t=2)
        xb2 = p_x.tile([P, KT1, L, 2], FP8, name="xb2")
        w1a2 = p_w1a.tile([P, KT1, M1, 2], FP8, name="w1a2")
        for k0, k1 in ((0, 2), (2, 4), (4, 8), (8, 12)):
            nc.sync.dma_start(w1a2[:, k0:k1, 0:P, :], w1a_r[:, k0:k1, 0:P, :])
            nc.sync.dma_start(xb2[:, k0:k1, 0:N1, :], xT_r[:, k0:k1, 0:N1, :])
        nc.sync.dma_start(w1a2[:, 12:KT1, 0:P, :], w1a_r[:, 12:KT1, 0:P, :])
        nc.sync.dma_start(w1a2[:, 0:8, P:2 * P, :], w1a_r[:, 0:8, P:2 * P, :])
        nc.sync.dma_start(xb2[:, 12:KT1, 0:N1, :], xT_r[:, 12:KT1, 0:N1, :])
        nc.sync.dma_start(w1a2[:, 8:KT1, P:2 * P, :],
                          w1a_r[:, 8:KT1, P:2 * P, :])
        nc.sync.dma_start(w1a2[:, :, 2 * P:3 * P, :],
                          w1a_r[:, :, 2 * P:3 * P, :])
        nc.sync.dma_start(cos_t[:], cs_d[0:DH // 2, :])
        nc.sync.dma_start(bias_t[:], mask_d[:])
        nc.sync.dma_start(w1a2[:, :, 3 * P:M1, :], w1a_r[:, :, 3 * P:M1, :])
        nc.sync.dma_start(sin_t[:], cs_d[DH // 2:P, :])
        for nci in range(1, NCH):
            nc.sync.dma_start(xb2[:, :, nci * N1:(nci + 1) * N1, :],
                              xT_r[:, :, nci * N1:(nci + 1) * N1, :])
        wuk2 = p_wuk.tile([P, KT3, HDL, 2], FP8, name="wuk2")
        nc.sync.dma_start(wuk2[:],
                          wuk.rearrange("(k p) (m t) -> p k m t", p=P, t=2))
        wuv2 = p_wuv.tile([P, KT3, HDL, 2], FP8, name="wuv2")
        nc.sync.dma_start(wuv2[:],
                          wuv.rearrange("(k p) (m t) -> p k m t", p=P, t=2))

        # ---------- phase 1a: [xrkT | c_kvT] = [Wkr | Wdkv].T @ x.T ----
        # mt order puts the rope row-block first so each chunk's rope can
        # run on Pool while the PE continues with the c_kv rows.
        MTO = [MT1 - 1] + list(range(MT1 - 1))
        for nci in range(NCH):
            cs = slice(nci * N1, (nci + 1) * N1)
            for sl, mt in enumerate(MTO):
                ps = p_ps13.tile([P, N1], F32, tag="g", name=f"ps1a_{nci}_{mt}")
                hl_chain(ps[:], w1a2, slice(sl * P, (sl + 1) * P),
                         xb2, cs, KT1)
                if mt == MT1 - 1:
                    nc.vector.tensor_scalar_mul(xrk_t[:, cs], ps[:],
                                                FP8_PRE / S1)
                    rope_chunk(xrk_t, kall[:, HL, :], "k", nci)
                else:
                    hl_split(nc.vector, ckv2[:, mt, cs, 0], ckv2[:, mt, cs, 1],
                             ps[:], SCK / S1)
        # w1b prefetch queued behind the x stream
        w1b2 = p_w1b.tile([P, KT1, M1, 2], FP8, name="w1b2")
        nc.sync.dma_start(w1b2[:],
                          w1b.rearrange("(k p) (m t) -> p k m t", p=P, t=2))
        es_w1a.close()

        # ---------- phase 3k: k_cT = Wuk_hg.T @ c_kvT -> fp8 plane ----
        for nci in range(NCH):
            cs = slice(nci * N1, (nci + 1) * N1)
            for h in range(HL):
                ps = p_ps13.tile([P, N1], F32, tag="g", name=f"ps3k_{nci}_{h}")
                hl_chain(ps[:], wuk2, slice(h * P, (h + 1) * P),
                         ckv2, cs, KT3)
                nc.vector.tensor_scalar_mul(kall[:, h, cs], ps[:],
                                            FP8_PRE / S3)

        # ---------- phase 1b: [xrqT | q_cT] = fusedWq.T @ x.T ----------
        qdest = [qall[:, h, :] for h in range(HL)] + [xrq_t[:]]
        for nci in range(NCH):
            cs = slice(nci * N1, (nci + 1) * N1)
            for sl, mt in enumerate(MTO):
                ps = p_ps13.tile([P, N1], F32, tag="g", name=f"ps1b_{nci}_{mt}")
                hl_chain(ps[:], w1b2, slice(sl * P, (sl + 1) * P),
                         xb2, cs, KT1)
                nc.vector.tensor_scalar_mul(qdest[mt][:, cs], ps[:],
                                            FP8_PRE / S1)
                if mt == MT1 - 1:
                    rope_chunk(xrq_t, qall[:, HL, :], "q", nci)
        es_ps13.close()
        es_x.close()

        # ---------- pools for the fused 1b + attention + out-proj --------
        es_wo = ExitStack()
        p_wo = es_wo.enter_context(tc.tile_pool(name="wop", bufs=1,
                                                side="right"))
        wo2 = p_wo.tile([P, HL, D, 2], FP8, name="wo2")
        nc.sync.dma_start(wo2[:],
                          wo.rearrange("(k p) (d t) -> p k d t", p=P, t=2))
        es_ctx = ExitStack()
        p_ctx = es_ctx.enter_context(tc.tile_pool(name="ctxp", bufs=1))
        ctx2 = p_ctx.tile([P, HL, L, 2], FP8, name="ctx2")
        es_p4 = ExitStack()
        p_sc = es_p4.enter_context(tc.tile_pool(name="scp", bufs=2,
                                                space="PSUM"))
        p_av = es_p4.enter_context(tc.tile_pool(name="avp", bufs=2,
                                                space="PSUM"))
        # shared ring for ph5 chains and (bitcast) transpose targets; they
        # never interleave inside one bank
        p_ch = es_p4.enter_context(tc.tile_pool(name="chp", bufs=2,
                                                space="PSUM"))
        p_e = es_p4.enter_context(tc.tile_pool(name="expp", bufs=16))
        p_sm = es_p4.enter_context(tc.tile_pool(name="smallp", bufs=8))
        p_st = es_p4.enter_context(tc.tile_pool(name="stagep", bufs=4))

        escale = float(SCALE / (FP8_PRE * FP8_PRE))

        # -- phase-5 blocks of query chunk n-1, interleaved as fillers --
        # stage a full [P, D] row-block in bf16, one out-DMA per mt (4 calls
        # per query chunk instead of 16 — HWDGE desc-gen is 625ns per call)
        ph5_pending = deque()
        ph5_stage = {}

        def emit_ph5_block(mt, nci, ps):
            hl_chain(ps, ctx2, slice(mt * P, (mt + 1) * P),
                     wo2, slice(nci * N1, (nci + 1) * N1), HL)
            if nci == 0:
                ph5_stage[mt] = p_st.tile([P, D], BF16, tag="stage",
                                          name=f"st_{mt}")
            stg = ph5_stage[mt]
            nc.vector.tensor_scalar_mul(
                stg[:, nci * N1:(nci + 1) * N1], ps, 1.0 / S5)
            if nci == NCH - 1:
                nc.sync.dma_start(out_d[mt * P:(mt + 1) * P, :],
                                  ph5_stage.pop(mt)[:])

        # 3v chains (v = c_kv @ Wuv, bf16 + ones col) run as qch0-hs0
        # fillers: AV first touches vaug at hs1, scores don't need it
        def gen_3v(mt):
            ps = p_ch.tile([P, N1], F32, tag="ch", name=f"psv_{mt}")
            hl_chain(ps[:], ckv2, slice(mt * P, (mt + 1) * P),
                     wuv2, slice(0, HDL), KT3)
            va = vaug_t[mt].rearrange("p (h c) -> p h c", c=DV)
            nc.vector.tensor_scalar_mul(
                va[:, :, 0:DH], ps.rearrange("p (h c) -> p h c", c=DH),
                1.0 / S3)
            nc.vector.memset(va[:, :, DH:DV], 1.0)

        v3_pending = deque(range(KB))

        def filler():
            if v3_pending:
                gen_3v(v3_pending.popleft())
            elif ph5_pending:
                mt, nci = ph5_pending.popleft()
                ps = p_ch.tile([P, N1], F32, tag="ch",
                               name=f"ps5_{mt}_{nci}")
                emit_ph5_block(mt, nci, ps[:])


        # ---------- fused attention + out-projection loop ----------
        et_t = {}              # (h, kbp) -> exp tile, two key-blocks wide
        av_t = {}              # (h, qc) -> psum chain tile [P, N1] (1 bank)

        def av_chain_seg(hp, qc, kbp, half, first, last):
            kb = 2 * kbp + half
            nc.tensor.matmul(
                av_t[(hp, qc)][:, 0:DV],
                et_t[(hp, kbp)][:, half, qc * P:(qc + 1) * P],
                vaug_t[kb][:, hp * DV:(hp + 1) * DV],
                start=first, stop=last)

        def norm(qch, hp, qc):
            pc = av_t.pop((hp, qc))
            rc = p_sm.tile([P, 1], F32, tag="recip", name=f"rc_{qch}_{hp}_{qc}")
            nc.vector.reciprocal(rc[:], pc[:, DH:DV])
            cn = p_sm.tile([P, DH], BF16, tag="cn", name=f"cn_{qch}_{hp}_{qc}")
            nc.vector.tensor_scalar_mul(cn[:], pc[:, 0:DH], rc[:])
            et_t[("cn", hp, qc)] = cn

        for qch in range(NCH):
            qs = slice(qch * N1, (qch + 1) * N1)
            for hs in range(HL + 1):
                # transposes for head hs-2 (deferred so the DVE norm of that
                # head has certainly drained); bitcast [P,N1]f32 chain slots
                # to bf16 for the [P,P] transpose target
                if hs >= 2 or (qch > 0 and hs == 0):
                    hp2 = hs - 2 if hs >= 2 else HL - 1
                    qq = qch if hs >= 2 else qch - 1
                    for qc in range(4):
                        q0 = qq * 4 + qc
                        cn = et_t.pop(("cn", hp2, qc))
                        pt = p_ch.tile([P, N1], F32, tag="ch",
                                       name=f"tp_{qq}_{hp2}_{q0}")
                        ptb = pt[:].bitcast(BF16)[:, 0:P]
                        nc.tensor.transpose(ptb, cn[:], ident[:])
                        hl_split(nc.vector,
                                 ctx2[:, hp2, q0 * P:(q0 + 1) * P, 0],
                                 ctx2[:, hp2, q0 * P:(q0 + 1) * P, 1],
                                 ptb, CTXPRE)
                hp = hs - 1
                if hs > 0:
                    # one PSUM bank per accumulation chain: a start=True in a
                    # bank poisons the whole 2KB zero-region granule, so two
                    # interleaved chains must never share a bank
                    for qc in range(2):
                        av_t[(hp, qc)] = p_av.tile(
                            [P, N1], F32, tag="av", name=f"av_{qch}_{hp}_{qc}")
                # sweep 1: scores/exp of head hs + AV chains qc0/qc1 of
                # hs-1. The AV consumption order is rotated by +2 key-block
                # pairs so the chain starts on already-drained exp tiles
                # (PSUM accumulation is commutative); SC is emitted after
                # the AV/filler work so a psum-slot wait never idles the PE.
                for i in range(KBP):
                    kbp = i
                    if hs > 0:
                        if i % 4 == 3 and hs < HL:
                            filler()
                        for half in range(2):
                            av_chain_seg(hp, 0, kbp, half, i == 0 and half == 0,
                                         i == KBP - 1 and half == 1)
                            av_chain_seg(hp, 1, kbp, half, i == 0 and half == 0,
                                         i == KBP - 1 and half == 1)
                    elif qch == 0:
                        filler()
                        filler()
                    else:
                        filler()
                    if hs < HL:
                        sc = p_sc.tile([P, 2, N1], F32, tag="sc",
                                       name=f"sc_{qch}_{hs}_{i}")
                        et = p_e.tile([P, 2, N1], BF16, tag="expT",
                                      name=f"et_{qch}_{hs}_{i}")
                        for half in range(2):
                            kb = 2 * i + half
                            nc.tensor.matmul(
                                sc[:, half, :],
                                kall[:, hs::(HL - hs), kb * P:(kb + 1) * P],
                                qall[:, hs::(HL - hs), qs],
                                start=True, stop=True, perf_mode=DR)
                        if mask_ones:
                            nc.scalar.activation(
                                et[:], sc[:],
                                mybir.ActivationFunctionType.Exp,
                                scale=escale)
                        else:
                            for half in range(2):
                                kb = 2 * i + half
                                nc.scalar.activation(
                                    et[:, half, :], sc[:, half, :],
                                    mybir.ActivationFunctionType.Exp,
                                    bias=bias_t[:, kb:kb + 1], scale=escale)
                        et_t[(hs, i)] = et
                # sweep 2: AV chains qc2/qc3 of head hs-1 + norms
                if hs > 0:
                    norm(qch, hp, 0)
                    norm(qch, hp, 1)
                    for qc in range(2, 4):
                        av_t[(hp, qc)] = p_av.tile(
                            [P, N1], F32, tag="av", name=f"av_{qch}_{hp}_{qc}")
                    for i in range(KBP):
                        for half in range(2):
                            av_chain_seg(hp, 2, i, half, i == 0 and half == 0,
                                         i == KBP - 1 and half == 1)
                            av_chain_seg(hp, 3, i, half, i == 0 and half == 0,
                                         i == KBP - 1 and half == 1)
                        if i % 4 == 1:
                            filler()
                    for kbp in range(KBP):
                        et_t.pop((hp, kbp))
                    norm(qch, hp, 2)
                    norm(qch, hp, 3)
            ph5_pending.extend((qch * 4 + mt, nci)
                               for mt in range(4) for nci in range(NCH))


        # flush: transposes of the last head + phase 5 for the last chunk;
        # alternate two psum rings (sc is free now) so chains double-buffer
        for qc in range(4):
            q0 = (NCH - 1) * 4 + qc
            cn = et_t.pop(("cn", HL - 1, qc))
            pt = p_ch.tile([P, N1], F32, tag="ch", name=f"tpf_{q0}")
            ptb = pt[:].bitcast(BF16)[:, 0:P]
            nc.tensor.transpose(ptb, cn[:], ident[:])
            hl_split(nc.vector,
                     ctx2[:, HL - 1, q0 * P:(q0 + 1) * P, 0],
                     ctx2[:, HL - 1, q0 * P:(q0 + 1) * P, 1],
                     ptb, CTXPRE)
        fi = 0
        while ph5_pending:
            mt, nci = ph5_pending.popleft()
            if fi % 2 == 0:
                ps = p_sc.tile([P, 2, N1], F32, tag="sc",
                               name=f"ps5f_{fi}")[:, 0, :]
            else:
                ps = p_ch.tile([P, N1], F32, tag="ch", name=f"ps5f_{fi}")[:]
            fi += 1
            emit_ph5_block(mt, nci, ps)

        es_p4.close()
        es_ctx.close()
        es_ckv.close()
        es_wukv.close()
        es_att.close()
        es_wo.close()
        es_rope.close()
        es_w1b.close()
        es_xr.close()
        es_tab.close()

    nc.compile()
    return nc


_CACHE = {}


def _get_nc(mask_ones=True):
    key = ("nc", mask_ones)
    if key not in _CACHE:
        _CACHE[key] = build_nc(mask_ones)
    return _CACHE[key]


def _split8(a, pre):
    """hi-lo e4m3 pair of pre*a, interleaved innermost: [d0, d1*2]."""
    s = np.asarray(a, np.float32) * np.float32(pre)
    hi = s.astype(F8NP)
    lo = (s - hi.astype(np.float32)).astype(F8NP)
    return np.ascontiguousarray(np.stack([hi, lo], axis=-1).reshape(
        a.shape[0], a.shape[1] * 2))


def _host_prep(x, attention_mask, W_DKV, W_DQ, W_UK, W_UV, W_UQ, W_KR, W_QR,
               W_O):
    f = np.float32
    bf = ml_dtypes.bfloat16
    x = np.asarray(x, f)
    attention_mask = np.asarray(attention_mask)
    W_DKV, W_DQ = np.asarray(W_DKV, f), np.asarray(W_DQ, f)
    W_UK, W_UV, W_UQ = np.asarray(W_UK, f), np.asarray(W_UV, f), np.asarray(W_UQ, f)
    W_KR, W_QR, W_O = np.asarray(W_KR, f), np.asarray(W_QR, f), np.asarray(W_O, f)

    perm = np.concatenate([np.arange(0, DH, 2), np.arange(1, DH, 2)])
    # column layout [rope | dkv]: the rope block is computed first on-chip
    w1a = _split8(np.concatenate([W_KR[:, perm], W_DKV], axis=1), WPRE)
    xhls = [_split8(x[b].T, XPRE) for b in range(B)]

    inv = 1.0 / (10000.0 ** (np.arange(0, DH, 2, dtype=f) / DH))
    freqs = np.arange(L, dtype=f)[:, None] * inv[None, :]
    rope = np.concatenate([np.sin(freqs), np.cos(freqs)], axis=-1).astype(f)
    csT = np.ascontiguousarray(np.concatenate(
        [rope[:, 1::2].T, rope[:, 0::2].T], axis=0).astype(bf))

    maskbs = []
    for b in range(B):
        bias = np.where(attention_mask[b] == 0, f(NEG), f(0.0)).astype(f)
        maskbs.append(np.ascontiguousarray(bias.reshape(KB, P).T))

    w1bs, wuks, wuvs, wos = [], [], [], []
    for hg in range(HG):
        cols = slice(hg * HDL, (hg + 1) * HDL)
        wq = W_DQ @ np.concatenate([W_QR[:, perm], W_UQ[:, cols]], axis=1)
        w1bs.append(_split8(wq, WPRE))
        wuks.append(_split8(W_UK[:, cols], WPRE))
        wuvs.append(_split8(W_UV[:, cols], WPRE))
        wos.append(_split8(W_O[hg * HDL:(hg + 1) * HDL, :], WPRE))

    in_maps = []
    for c in range(8):
        b, hg = c // HG, c % HG
        in_maps.append({
            "xhl": xhls[b],
            "w1a": w1a,
            "w1b": w1bs[hg],
            "wuk": wuks[hg],
            "wuv": wuvs[hg],
            "wo": wos[hg],
            "csT": csT,
            "maskb": maskbs[b],
        })
    return in_maps


def kernel(x, attention_mask, W_DKV, W_DQ, W_UK, W_UV, W_UQ, W_KR, W_QR, W_O,
           **run_kwargs):
    in_maps = _host_prep(x, attention_mask, W_DKV, W_DQ, W_UK, W_UV, W_UQ,
                         W_KR, W_QR, W_O)
    mask_ones = bool(np.all(np.asarray(attention_mask) != 0))
    nc = _get_nc(mask_ones)
    res = run_bass_kernel_spmd(nc, in_maps, core_ids=list(range(8)),
                               **run_kwargs)
    out = np.zeros((B, L, D), np.float32)
    for c in range(8):
        out[c // HG] += res.results[c]["out"].astype(np.float32)
    if run_kwargs:
        _CACHE["last_results"] = res
    return out
